# revision 13
# baseline (speedup 1.0000x reference)
"""GCNII forward on 8 TRN2 NeuronCores (self-contained).

Strategy (balanced 1D row partitioning, section-pipelined exchange):
- nodes assigned to 160 (core,tile,slot) buckets round-robin by in-degree so
  every dst tile sees ~E/160 edges; output rows un-permuted on the host.
- exchange table in fp8e4 (dinv*h), split into TWO section tensors:
  sec1 = tiles 0-11 (AllGathered right after tile 11's back phase, hidden
  under tiles 12-19), sec2 = tiles 12-19 (AllGathered at layer end).
- per-tile edges sorted by source section; each tile does 2 batched
  dma_gathers (sec1 chunks, sec2 chunks) -> SWDGE per-call overhead 2x/tile
  instead of 4x.
- first NS tiles of each layer run a split spmm: sec1 chunks aggregate+stash
  to SBUF (bf16) while the sec2 AllGather is still in flight, then sec2
  chunks complete the aggregate -> the inter-layer AllGather bubble is
  overlapped with useful work.
- scatter-add via one-hot fp8 DoubleRow matmuls; self-loop term handled
  locally on ACT/DVE from the SBUF-resident fp8 e table.
- z transposed for the layer GEMM via PE identity transposes, GEMM in fp8
  DoubleRow: h = relu((1-b)*z + (b/s)*(q8(z) @ q8(s*b*Wg))), s a power of 2.
- h0 residual (0.1*h0) kept in SBUF as bf16; phase0 GEMM in bf16.
"""
import numpy as np
from ml_dtypes import float8_e4m3, bfloat16

import concourse.bass as bass
import concourse.mybir as mybir
import concourse.tile as tile
from concourse import bacc
from concourse.bass_utils import run_bass_kernel_spmd
from concourse.masks import make_identity

N, E = 20000, 320000
F_IN, H, C, L = 512, 1024, 64, 8
ALPHA, THETA = 0.1, 0.5
NCORES = 8
P = 128
SHP = 2560                  # padded rows per core (20*128)
NT = SHP // P               # 20 dst tiles per core
S1T, S2T = 12, 8            # tiles per table section
R1, R2 = S1T * P, S2T * P   # rows per core per section
V1, V2 = NCORES * R1, NCORES * R2
NS = 0                      # stash (split-spmm) tiles per layer
NPB = 125                   # real nodes per bucket (20000/160)
KF = F_IN // P
KH = H // P

f32 = mybir.dt.float32
bf16 = mybir.dt.bfloat16
fp8 = mybir.dt.float8e4
i16 = mybir.dt.int16

BETAS = np.log(THETA / np.arange(1.0, L + 1.0) + 1.0).astype(np.float64)

_cache = {}


def _evenceil(x):
    c = (x + P - 1) // P
    return c + (c % 2)


def _preprocess(x, edge_index, W1, b1, Wg, W2, b2):
    src = np.asarray(edge_index[0], dtype=np.int64)
    dst = np.asarray(edge_index[1], dtype=np.int64)
    deg = (np.bincount(dst, minlength=N) + 1).astype(np.float32)  # +self
    dinv = 1.0 / np.sqrt(deg)

    # balanced assignment: nodes sorted by in-degree, snake round-robin over
    # the 160 buckets; node -> (bucket, slot)
    nb = NCORES * NT
    order = np.argsort(-deg, kind="stable")
    bucket = np.zeros(N, dtype=np.int64)
    slot = np.zeros(N, dtype=np.int64)
    pos = np.arange(N)
    rnd = pos // nb
    off = pos % nb
    snake = np.where(rnd % 2 == 0, off, nb - 1 - off)
    bucket[order] = snake
    slot[order] = rnd
    assert slot.max() == NPB - 1
    core_of = bucket // NT
    tile_of = bucket % NT
    loc = tile_of * P + slot                    # row within core [0, 2560)
    sec_of = (tile_of >= S1T).astype(np.int64)  # source section
    # row within the section table
    trow = np.where(sec_of == 0,
                    core_of * R1 + loc,
                    core_of * R2 + (loc - R1))

    # edges grouped by (dst bucket, src section)
    gid = bucket[dst]
    key = gid * 2 + sec_of[src]
    eorder = np.argsort(key, kind="stable")
    key_s = key[eorder]
    gid_s = gid[eorder]
    rows_s = trow[src[eorder]]
    dslot_s = slot[dst[eorder]]
    kcnt = np.bincount(key_s, minlength=nb * 2)
    kstarts = np.concatenate([[0], np.cumsum(kcnt)[:-1]])
    j = np.arange(len(key_s)) - kstarts[key_s]

    cnt1 = kcnt[0::2].reshape(NCORES, NT)
    cnt2 = kcnt[1::2].reshape(NCORES, NT)
    c1_t = _evenceil(cnt1.max(0))               # [NT] chunks (sec1), even
    c2_t = -(-cnt2.max(0) // P)                 # [NT] chunks (sec2)
    ctot_t = c1_t + c2_t
    TOTC = int(ctot_t.sum())
    base = np.zeros(NT, dtype=np.int64)
    base[1:] = np.cumsum(ctot_t)[:-1]

    core_s = gid_s // NT
    tl_s = gid_s % NT
    sec_s = key_s % 2
    gcol = base[tl_s] + np.where(sec_s == 0, 0, c1_t[tl_s]) + j // P
    p_idx = j % P
    flat = gcol * P + p_idx
    icol = flat // 16
    irow = flat % 16
    idx16 = np.zeros((NCORES, 128, TOTC * 8), dtype=np.int16)
    S = np.zeros((NCORES, P, TOTC, P), dtype=float8_e4m3)
    for r in range(8):
        idx16[core_s, r * 16 + irow, icol] = rows_s.astype(np.int16)
    S[core_s, p_idx, gcol, dslot_s] = 1.0

    # per-core padded dinv columns [P, NT] (0 on pad slots)
    dpad = np.zeros((NCORES, NT, P), dtype=np.float32)
    dpad[core_of, tile_of, slot] = dinv
    dcols = dpad.transpose(0, 2, 1).copy()      # [c, P, NT]
    d09 = (0.9 * dcols).astype(np.float32)
    escale = np.zeros((NCORES, P, L * NT), dtype=np.float32)
    for l in range(L):
        escale[:, :, l * NT:(l + 1) * NT] = (1.0 - BETAS[l]) * dcols

    # x packed for lhsT: xp[c, t, p, k*128+m] = x[node(c,t,m), k*128+p]
    x = np.asarray(x, dtype=np.float32)
    xsh = np.zeros((NCORES, NT, P, F_IN), dtype=np.float32)
    xsh[core_of, tile_of, slot] = x
    xp = np.ascontiguousarray(
        xsh.reshape(NCORES, NT, P, KF, P).transpose(0, 1, 4, 3, 2)
    ).reshape(NCORES, NT, P, F_IN).astype(bfloat16)

    def pack_w(w, dt):  # [K, Nout] -> [P, K//P, Nout]
        K = w.shape[0]
        return np.ascontiguousarray(
            w.reshape(K // P, P, -1).transpose(1, 0, 2)).astype(dt)

    W1p = pack_w(np.asarray(W1, np.float32), bfloat16)
    W2p = pack_w(np.asarray(W2, np.float32), bfloat16)
    Wg = np.asarray(Wg, dtype=np.float64)
    scales = []
    Wqs = []
    for l in range(L):
        bw = BETAS[l] * Wg[l]
        s = 2.0 ** np.floor(np.log2(240.0 / np.abs(bw).max()))
        scales.append(float(s))
        Wqs.append(pack_w((s * bw).astype(np.float32), float8_e4m3))
    Wq = np.stack(Wqs)

    b1b = np.broadcast_to(np.asarray(b1, np.float32), (P, H)).copy()
    b2b = np.broadcast_to(np.asarray(b2, np.float32), (P, C)).copy()

    meta = {"c1": c1_t, "c2": c2_t, "base": base, "TOTC": TOTC,
            "scales": scales, "core_of": core_of, "loc": loc}
    in_maps = []
    for c in range(NCORES):
        in_maps.append({
            "xp": xp[c],
            "W1p": W1p, "W2p": W2p, "Wq": Wq,
            "b1b": b1b, "b2b": b2b,
            "d09": d09[c], "dinvc": dcols[c].astype(np.float32),
            "escale": escale[c],
            "idx16": idx16[c], "Smat": S[c],
        })
    return in_maps, meta


def _build(meta):
    TOTC = meta["TOTC"]
    c1_t = meta["c1"]
    c2_t = meta["c2"]
    base0 = meta["base"]
    scales = meta["scales"]
    C1M = int(c1_t.max())
    C2M = int(c2_t.max())

    nc = bacc.Bacc("TRN2", target_bir_lowering=False, debug=False,
                   num_devices=NCORES, num_swdge_queues=4)
    t_xp = nc.dram_tensor("xp", [NT, P, F_IN], bf16, kind="ExternalInput")
    t_W1 = nc.dram_tensor("W1p", [P, KF, H], bf16, kind="ExternalInput")
    t_W2 = nc.dram_tensor("W2p", [P, KH, C], bf16, kind="ExternalInput")
    t_Wq = nc.dram_tensor("Wq", [L, P, KH, H], fp8, kind="ExternalInput")
    t_b1 = nc.dram_tensor("b1b", [P, H], f32, kind="ExternalInput")
    t_b2 = nc.dram_tensor("b2b", [P, C], f32, kind="ExternalInput")
    t_d09 = nc.dram_tensor("d09", [P, NT], f32, kind="ExternalInput")
    t_dinv = nc.dram_tensor("dinvc", [P, NT], f32, kind="ExternalInput")
    t_esc = nc.dram_tensor("escale", [P, L * NT], f32, kind="ExternalInput")
    t_idx = nc.dram_tensor("idx16", [128, TOTC * 8], i16, kind="ExternalInput")
    t_S = nc.dram_tensor("Smat", [P, TOTC, P], fp8, kind="ExternalInput")
    t_out = nc.dram_tensor("out", [SHP, C], f32, kind="ExternalOutput")

    exch = nc.dram_tensor("exch", [SHP, H], fp8)
    tbl1 = [nc.dram_tensor(f"tbl1_{i}", [V1, H], fp8, addr_space="Shared")
            for i in range(2)]
    tbl2 = [nc.dram_tensor(f"tbl2_{i}", [V2, H], fp8, addr_space="Shared")
            for i in range(2)]

    DR = mybir.MatmulPerfMode.DoubleRow
    ACT = mybir.ActivationFunctionType

    def ag1(dst_tbl):
        nc.gpsimd.collective_compute(
            "AllGather", mybir.AluOpType.bypass,
            replica_groups=[list(range(NCORES))],
            ins=[exch.ap()[0:R1].opt()], outs=[dst_tbl.ap().opt()])

    def ag2(dst_tbl):
        nc.gpsimd.collective_compute(
            "AllGather", mybir.AluOpType.bypass,
            replica_groups=[list(range(NCORES))],
            ins=[exch.ap()[R1:SHP].opt()], outs=[dst_tbl.ap().opt()])

    with tile.TileContext(nc) as tc:
        with (
            tc.tile_pool(name="const", bufs=1) as cp,
            tc.tile_pool(name="wpool", bufs=2) as wp,
            tc.tile_pool(name="xpool", bufs=2) as xp_,
            tc.tile_pool(name="gpool", bufs=2) as gp,
            tc.tile_pool(name="zpool", bufs=2) as zp,
            tc.tile_pool(name="ps_agg", bufs=2, space="PSUM") as pa,
            tc.tile_pool(name="ps_gemm", bufs=1, space="PSUM") as pg,
            tc.tile_pool(name="ps_tr", bufs=2, space="PSUM") as pt,
        ):
            ident = cp.tile([P, P], f32, tag="ident")
            make_identity(nc, ident[:])
            idx_sb = cp.tile([128, TOTC * 8], i16, tag="idx")
            nc.sync.dma_start(out=idx_sb[:], in_=t_idx[:])
            S_sb = cp.tile([P, TOTC, P], fp8, tag="S")
            nc.sync.dma_start(out=S_sb[:], in_=t_S[:])
            d09_sb = cp.tile([P, NT], f32, tag="d09")
            nc.sync.dma_start(out=d09_sb[:], in_=t_d09[:])
            dinv_sb = cp.tile([P, NT], f32, tag="dinv")
            nc.sync.dma_start(out=dinv_sb[:], in_=t_dinv[:])
            esc_sb = cp.tile([P, L * NT], f32, tag="esc")
            nc.sync.dma_start(out=esc_sb[:], in_=t_esc[:])
            b1_sb = cp.tile([P, H], f32, tag="b1")
            nc.sync.dma_start(out=b1_sb[:], in_=t_b1[:])
            b2_sb = cp.tile([P, C], f32, tag="b2")
            nc.sync.dma_start(out=b2_sb[:], in_=t_b2[:])
            W1_sb = cp.tile([P, KF, H], bf16, tag="W1")
            nc.sync.dma_start(out=W1_sb[:], in_=t_W1[:])
            W2_sb = cp.tile([P, KH, C], bf16, tag="W2")
            nc.sync.dma_start(out=W2_sb[:], in_=t_W2[:])
            h0s_sb = cp.tile([P, NT, H], bf16, tag="h0s")
            e_sb = cp.tile([P, NT, H], fp8, tag="e")
            zst_sb = cp.tile([P, max(NS, 1), H], bf16, tag="zst")

            # ---- phase 0
            for t in range(NT):
                xt = xp_.tile([P, KF, P], bf16, tag="xt")
                nc.sync.dma_start(out=xt[:], in_=t_xp[t])
                ps = pg.tile([P, H], f32, space="PSUM", tag="gemm")
                for k in range(KF):
                    for nh in range(2):
                        nc.tensor.matmul(
                            out=ps[:, nh * 512:(nh + 1) * 512],
                            lhsT=xt[:, k, :],
                            rhs=W1_sb[:, k, nh * 512:(nh + 1) * 512],
                            start=(k == 0), stop=(k == KF - 1))
                nc.vector.tensor_add(out=ps[:], in0=ps[:], in1=b1_sb[:])
                nc.scalar.activation(out=h0s_sb[:, t, :], in_=ps[:],
                                     func=ACT.Relu, scale=0.1)
                nc.scalar.activation(out=e_sb[:, t, :], in_=ps[:],
                                     func=ACT.Relu,
                                     scale=dinv_sb[:, t:t + 1])
                nc.sync.dma_start(out=exch[t * P:(t + 1) * P, :],
                                  in_=e_sb[:, t, :])
                if t == S1T - 1:
                    ag1(tbl1[0])
            ag2(tbl2[0])

            # ---- layers
            qctr = [0]
            for l in range(L):
                tb1, tb2 = tbl1[l % 2], tbl2[l % 2]
                beta = float(BETAS[l])
                cprime = beta / ((1.0 - beta) * scales[l])
                Wq_sb = wp.tile([P, KH, H], fp8, tag="W")
                nc.sync.dma_start(out=Wq_sb[:], in_=t_Wq[l])

                GMAX = 4        # chunks per dma_gather call

                def gather1(t):
                    c1 = int(c1_t[t])
                    b8 = int(base0[t]) * 8
                    gA = gp.tile([P, C1M, H], fp8, tag="gA", name="gA")
                    for cc0 in range(0, c1, GMAX):
                        w = min(GMAX, c1 - cc0)
                        nc.gpsimd.dma_gather(
                            out_ap=gA[:, cc0:cc0 + w, :], in_ap=tb1.ap(),
                            idxs_ap=idx_sb[:, b8 + cc0 * 8:b8 + (cc0 + w) * 8],
                            num_idxs=w * P, num_idxs_reg=w * P,
                            elem_size=H, queue_num=qctr[0] % 4)
                        qctr[0] += 1
                    return gA

                def gather2(t):
                    c1, c2 = int(c1_t[t]), int(c2_t[t])
                    b8 = (int(base0[t]) + c1) * 8
                    gB = gp.tile([P, C2M, H], fp8, tag="gB", name="gB")
                    for cc0 in range(0, c2, GMAX):
                        w = min(GMAX, c2 - cc0)
                        nc.gpsimd.dma_gather(
                            out_ap=gB[:, cc0:cc0 + w, :], in_ap=tb2.ap(),
                            idxs_ap=idx_sb[:, b8 + cc0 * 8:b8 + (cc0 + w) * 8],
                            num_idxs=w * P, num_idxs_reg=w * P,
                            elem_size=H, queue_num=qctr[0] % 4)
                        qctr[0] += 1
                    return gB

                def mm_pairs(agg, g, t, cols, s_off, start, stop):
                    bS = int(base0[t]) + s_off
                    npair = cols // 2
                    tail = cols % 2
                    for kp in range(npair):
                        for nh in range(2):
                            nc.tensor.matmul(
                                out=agg[:, nh * 512:(nh + 1) * 512],
                                lhsT=S_sb[:, bS + 2 * kp:bS + 2 * kp + 2, :],
                                rhs=g[:, 2 * kp:2 * kp + 2,
                                      nh * 512:(nh + 1) * 512],
                                start=(start and kp == 0),
                                stop=(stop and not tail and kp == npair - 1),
                                perf_mode=DR)
                    if tail:
                        for nh in range(2):
                            nc.tensor.matmul(
                                out=agg[:, nh * 512:(nh + 1) * 512],
                                lhsT=S_sb[:, bS + cols - 1, :],
                                rhs=g[:, cols - 1, nh * 512:(nh + 1) * 512],
                                start=(start and npair == 0),
                                stop=stop)

                def front_A(t):
                    """sec1 gather + aggregate, stashed to SBUF (bf16)."""
                    gA = gather1(t)
                    aggA = pa.tile([P, H], f32, space="PSUM", tag="agg",
                                   name="aggA")
                    mm_pairs(aggA, gA, t, int(c1_t[t]), 0, True, True)
                    nc.scalar.activation(out=zst_sb[:, t, :], in_=aggA[:],
                                         func=ACT.Copy,
                                         scale=d09_sb[:, t:t + 1])

                def front_B(t):
                    """sec2 gather + aggregate + combine with stash -> z."""
                    gB = gather2(t)
                    vs = zp.tile([P, H], f32, tag="vs", name="vs")
                    nc.scalar.activation(out=vs[:], in_=e_sb[:, t, :],
                                         func=ACT.Copy,
                                         scale=d09_sb[:, t:t + 1])
                    nc.vector.tensor_add(out=vs[:], in0=vs[:],
                                         in1=h0s_sb[:, t, :])
                    aggB = pa.tile([P, H], f32, space="PSUM", tag="agg",
                                   name="aggB")
                    mm_pairs(aggB, gB, t, int(c2_t[t]), int(c1_t[t]),
                             True, True)
                    z = zp.tile([P, H], f32, tag="z", name="z")
                    nc.scalar.activation(out=z[:], in_=aggB[:], func=ACT.Copy,
                                         scale=d09_sb[:, t:t + 1])
                    nc.vector.tensor_add(out=z[:], in0=z[:],
                                         in1=zst_sb[:, t, :])
                    nc.vector.tensor_add(out=z[:], in0=z[:], in1=vs[:])
                    return z

                def front_full(t):
                    """both gathers + one aggregate; returns z."""
                    gA = gather1(t)
                    gB = gather2(t)
                    vs = zp.tile([P, H], f32, tag="vs", name="vs")
                    nc.scalar.activation(out=vs[:], in_=e_sb[:, t, :],
                                         func=ACT.Copy,
                                         scale=d09_sb[:, t:t + 1])
                    nc.vector.tensor_add(out=vs[:], in0=vs[:],
                                         in1=h0s_sb[:, t, :])
                    agg = pa.tile([P, H], f32, space="PSUM", tag="agg",
                                  name="agg")
                    mm_pairs(agg, gA, t, int(c1_t[t]), 0, True, False)
                    mm_pairs(agg, gB, t, int(c2_t[t]), int(c1_t[t]),
                             False, True)
                    z = zp.tile([P, H], f32, tag="z", name="z")
                    nc.scalar.activation(out=z[:], in_=agg[:], func=ACT.Copy,
                                         scale=d09_sb[:, t:t + 1])
                    nc.vector.tensor_add(out=z[:], in0=z[:], in1=vs[:])
                    return z

                def gemm_zW(z):
                    """transpose z, fp8 GEMM with Wq; returns PSUM ps2."""
                    zqT = zp.tile([P, KH, P], fp8, tag="zqT", bufs=1,
                                  name="zqT")
                    for k in range(KH):
                        trp = pt.tile([P, P], f32, space="PSUM", tag="tr",
                                      name="trp")
                        nc.tensor.transpose(out=trp[:],
                                            in_=z[:, k * P:(k + 1) * P],
                                            identity=ident[:])
                        nc.vector.tensor_copy(out=zqT[:, k, :], in_=trp[:])
                    ps2 = pg.tile([P, H], f32, space="PSUM", tag="gemm",
                                  name="ps2")
                    for kp in range(KH // 2):
                        for nh in range(2):
                            nc.tensor.matmul(
                                out=ps2[:, nh * 512:(nh + 1) * 512],
                                lhsT=zqT[:, 2 * kp:2 * kp + 2, :],
                                rhs=Wq_sb[:, 2 * kp:2 * kp + 2,
                                          nh * 512:(nh + 1) * 512],
                                start=(kp == 0), stop=(kp == KH // 2 - 1),
                                perf_mode=DR)
                    return ps2

                def tile_back(t, z):
                    ps2 = gemm_zW(z)
                    w = zp.tile([P, H], f32, tag="w", name="w")
                    nc.scalar.activation(out=w[:], in_=ps2[:], func=ACT.Copy,
                                         scale=cprime)
                    nc.vector.tensor_add(out=w[:], in0=w[:], in1=z[:])
                    nc.scalar.activation(
                        out=e_sb[:, t, :], in_=w[:], func=ACT.Relu,
                        scale=esc_sb[:, l * NT + t:l * NT + t + 1])
                    nc.sync.dma_start(out=exch[t * P:(t + 1) * P, :],
                                      in_=e_sb[:, t, :])
                    if t == S1T - 1:
                        ag1(tbl1[(l + 1) % 2])

                def tile_back_final(t, z):
                    ps2 = gemm_zW(z)
                    w = zp.tile([P, H], f32, tag="w", name="w")
                    nc.scalar.activation(out=w[:], in_=ps2[:], func=ACT.Copy,
                                         scale=cprime)
                    nc.vector.tensor_add(out=w[:], in0=w[:], in1=z[:])
                    h8 = zp.tile([P, H], f32, tag="vs")
                    nc.scalar.activation(out=h8[:], in_=w[:],
                                         func=ACT.Relu,
                                         scale=1.0 - beta)
                    h8T = zp.tile([P, KH, P], bf16, tag="h8T", bufs=1)
                    for k in range(KH):
                        trp = pt.tile([P, P], f32, space="PSUM", tag="tr")
                        nc.tensor.transpose(out=trp[:],
                                            in_=h8[:, k * P:(k + 1) * P],
                                            identity=ident[:])
                        nc.vector.tensor_copy(out=h8T[:, k, :], in_=trp[:])
                    psl = pt.tile([P, P], f32, space="PSUM", tag="tr")
                    for k in range(KH):
                        nc.tensor.matmul(
                            out=psl[:, 0:C],
                            lhsT=h8T[:, k, :],
                            rhs=W2_sb[:, k, :],
                            start=(k == 0), stop=(k == KH - 1))
                    nc.vector.tensor_add(out=psl[:, 0:C], in0=psl[:, 0:C],
                                         in1=b2_sb[:])
                    mx = zp.tile([P, 1], f32, tag="mx")
                    nc.vector.tensor_reduce(out=mx[:], in_=psl[:, 0:C],
                                            axis=mybir.AxisListType.X,
                                            op=mybir.AluOpType.max)
                    nmx = zp.tile([P, 1], f32, tag="nmx")
                    nc.vector.tensor_scalar(
                        out=nmx[:], in0=mx[:], scalar1=-1.0, scalar2=None,
                        op0=mybir.AluOpType.mult)
                    esb = zp.tile([P, C], f32, tag="esb")
                    se = zp.tile([P, 1], f32, tag="se")
                    nc.scalar.activation(out=esb[:], in_=psl[:, 0:C],
                                         func=ACT.Exp,
                                         bias=nmx[:], accum_out=se[:])
                    lse = zp.tile([P, 1], f32, tag="lse")
                    nc.scalar.activation(out=lse[:], in_=se[:],
                                         func=ACT.Ln)
                    o_t = zp.tile([P, C], f32, tag="ot")
                    nc.vector.tensor_scalar(
                        out=o_t[:], in0=psl[:, 0:C], scalar1=mx[:],
                        scalar2=lse[:],
                        op0=mybir.AluOpType.subtract,
                        op1=mybir.AluOpType.subtract)
                    nc.sync.dma_start(out=t_out[t * P:(t + 1) * P, :],
                                      in_=o_t[:])

                back = tile_back if l < L - 1 else tile_back_final
                # stash prologue: sec1 work for the first NS tiles runs
                # while the sec2 AllGather is still in flight
                for t in range(NS):
                    front_A(t)
                zprev = None
                for t in range(NT):
                    zc = front_B(t) if t < NS else front_full(t)
                    if zprev is not None:
                        back(t - 1, zprev)
                    zprev = zc
                back(NT - 1, zprev)
                if l < L - 1:
                    ag2(tbl2[(l + 1) % 2])
    nc.compile()
    return nc


def kernel(**inputs):
    in_maps, meta = _preprocess(
        inputs["x"], inputs["edge_index"], inputs["W1"], inputs["b1"],
        inputs["Wg"], inputs["W2"], inputs["b2"])
    key = ("nc", meta["TOTC"], tuple(meta["c1"]), tuple(meta["c2"]),
           tuple(meta["scales"]))
    if key not in _cache:
        _cache[key] = _build(meta)
    nc = _cache[key]
    res = run_bass_kernel_spmd(nc, in_maps, list(range(NCORES)))
    per_core = np.stack([res.results[c]["out"] for c in range(NCORES)])
    out = per_core[meta["core_of"], meta["loc"]]
    return out.astype(np.float32)
